# revision 1
# baseline (speedup 1.0000x reference)
"""Trainium2 Bass kernel for nn_EnhancedFractionalPINO.

Pipeline (per core, batch-parallel over 8 NeuronCores, 32 batches/core):
  1. f = Re(fft2(x)) per 64x64 image via cosine/sine DFT matmuls:
     m1: per image, lhsT = image, rhs = [C | S] -> [x^T C | x^T S];
     m2: per 8-image group, two const-stationary matmuls with strided rhs
     -> A^T = C x^T C - S x^T S for all 8 images in one psum tile.
  2. GL fractional derivative = truncated causal conv (KTAPS taps) over the
     globally-flattened signal, as Toeplitz-block matmuls (halo image passed
     from the previous core's batch range; zeros for core 0). The h^-alpha
     scale is folded into Ws1 so everything stays in fp16 range.
  3. spectral_operator + neural_operator MLPs as fp16 PE matmuls with a
     positive rescaling chain (LAM_*) keeping activations in fp16 range;
     activations-stationary, PE transposes between layers.
  4. out = Re(ifft2(proc)) via the same DFT-matmul machinery (scales folded
     into the second-stage constants).

Weights are replicated across cores; activations stay SBUF-resident.
"""

import numpy as np

import concourse.bass as bass
import concourse.mybir as mybir
import concourse.tile as tile
from concourse import bacc
from concourse.bass_utils import run_bass_kernel_spmd

F32 = mybir.dt.float32
F16 = mybir.dt.float16
AF = mybir.ActivationFunctionType

B, C, H, W = 256, 3, 64, 64
MODES = C * H * W              # 12288
ALPHA = 0.5
NTOT = B * MODES               # 3145728 flattened samples
NCORE = 8
BS = B // NCORE                # 32 batches per core
NIMG = BS * C                  # 96 images per core
NSLOT = NIMG + 2               # halo + 96 images + zero pad
KTAPS = 512                    # truncated GL taps (4 chunks of 128)
NCH = BS * MODES // 128        # 3072 output chunks per core
NBLK = NCH // 512              # 6 conv blocks of 512 chunks

# fp16 activation rescaling chain (see mirror3 validation)
LAM_H, LAM_S, LAM_1, LAM_2, LAM_P = 16.0, 8.0, 4.0, 4.0, 4.0


# ---------------------------------------------------------------- host consts
def _host_constants():
    jk = np.outer(np.arange(64), np.arange(64)).astype(np.float64)
    Cm = np.cos(2 * np.pi * jk / 64)
    Sm = np.sin(2 * np.pi * jk / 64)

    j = np.arange(1, KTAPS, dtype=np.float64)
    w = np.concatenate([[1.0], np.cumprod((j - 1.0 - ALPHA) / j)])

    # Tst[d][t, tau] = w[128*d + tau - t]  (lhsT layout of the Toeplitz blocks)
    idx = 128 * np.arange(4)[:, None, None] \
        + np.arange(128)[None, None, :] - np.arange(128)[None, :, None]
    Tst = np.where((idx >= 0) & (idx < KTAPS), w[np.clip(idx, 0, KTAPS - 1)], 0.0)

    f16 = lambda a: np.ascontiguousarray(a, dtype=np.float16)
    return {
        "cswi": f16(np.concatenate([Cm, Sm], axis=1)),     # [64, 128]
        "cmf": f16(Cm),                                    # [64, 64]
        "msf": f16(-Sm),
        "cmi": f16(Cm * (LAM_P / 4096.0)),
        "smi": f16(-Sm * (LAM_P / 4096.0)),
        "tst": f16(Tst),
        "idn32": f16(np.eye(32)),
        "ones1": f16(np.ones((1, 32))),
    }


def _prep_weights(Ws1, bs1, Ws2, bs2, Wn1, bn1, Wn2, bn2, Wn3, bn3):
    s = float(np.float64(1.0 / (NTOT - 1)) ** (-ALPHA))
    f16 = lambda a: np.ascontiguousarray(a, dtype=np.float16)
    W1 = (Ws1.astype(np.float64) * (s / LAM_H)).astype(np.float32)
    W2 = Ws2 * np.float32(LAM_H / LAM_S)
    W3 = Wn1 * np.float32(LAM_S / LAM_1)
    W4 = Wn2 * np.float32(LAM_1 / LAM_2)
    W5 = Wn3 * np.float32(LAM_2 / LAM_P)
    return {
        "w1t": f16(W1.reshape(24, 4, 128, 512).transpose(0, 2, 1, 3)),
        "w2r": f16(W2.reshape(4, 128, 12, 1024).transpose(2, 1, 0, 3)),
        "w3t": f16(W3.reshape(24, 4, 128, 512).transpose(0, 2, 1, 3)),
        "w4t": f16(W4.reshape(4, 128, 4, 128).transpose(2, 1, 0, 3)
                   .reshape(4, 128, 512)),
        "w5r": f16(W5.reshape(4, 128, 12, 1024).transpose(2, 1, 0, 3)),
        "b1r": f16((bs1 / LAM_H).reshape(1, 512)),
        "b2r": f16((bs2 / LAM_S).reshape(1, MODES)),
        "b3r": f16((bn1 / LAM_1).reshape(1, 512)),
        "b4t": np.ascontiguousarray((bn2 / LAM_2).reshape(4, 128).T,
                                    dtype=np.float32),     # [128, 4]
        "b5r": f16((bn3 / LAM_P).reshape(1, MODES)),
    }


# ---------------------------------------------------------------- bass module
_NC_CACHE = None


def _build_nc():
    nc = bacc.Bacc("TRN2", target_bir_lowering=False, debug=False,
                   num_devices=NCORE)

    def din(name, shape, dt=F16):
        return nc.dram_tensor(name, shape, dt, kind="ExternalInput")

    d_x = din("ximgs", (NSLOT, 64, 64))
    d_cswi = din("cswi", (64, 128))
    d_cmf = din("cmf", (64, 64))
    d_msf = din("msf", (64, 64))
    d_cmi = din("cmi", (64, 64))
    d_smi = din("smi", (64, 64))
    d_tst = din("tst", (4, 128, 128))
    d_idn = din("idn32", (32, 32))
    d_ones = din("ones1", (1, 32))
    d_w1 = din("w1t", (24, 128, 4, 512))
    d_w2 = din("w2r", (12, 128, 4, 1024))
    d_w3 = din("w3t", (24, 128, 4, 512))
    d_w4 = din("w4t", (4, 128, 512))
    d_w5 = din("w5r", (12, 128, 4, 1024))
    d_b1 = din("b1r", (1, 512))
    d_b2 = din("b2r", (1, MODES))
    d_b3 = din("b3r", (1, 512))
    d_b4 = nc.dram_tensor("b4t", (128, 4), F32, kind="ExternalInput")
    d_b5 = din("b5r", (1, MODES))
    d_out = nc.dram_tensor("out", (BS, C, 64, 64), F32, kind="ExternalOutput")

    with tile.TileContext(nc) as tc:
        with tc.tile_pool(name="cpool", bufs=1) as cpool, \
             tc.tile_pool(name="bigpool", bufs=1) as bigpool:
            # ---- constants into SBUF
            cswi = cpool.tile([64, 128], F16, tag="cswi")
            cmf = cpool.tile([64, 64], F16, tag="cmf")
            msf = cpool.tile([64, 64], F16, tag="msf")
            cmi = cpool.tile([64, 64], F16, tag="cmi")
            smi = cpool.tile([64, 64], F16, tag="smi")
            tsb = cpool.tile([128, 4, 128], F16, tag="tsb")
            idn = cpool.tile([32, 32], F16, tag="idn")
            ones1 = cpool.tile([1, 32], F16, tag="ones1")
            b1s = cpool.tile([1, 512], F16, tag="b1s")
            b3s = cpool.tile([1, 512], F16, tag="b3s")
            b4s = cpool.tile([128, 4], F32, tag="b4s")
            bbig = cpool.tile([1, MODES], F16, tag="bbig")  # b2 then b5
            for t, d in ((cswi, d_cswi), (cmf, d_cmf), (msf, d_msf),
                         (cmi, d_cmi), (smi, d_smi), (idn, d_idn),
                         (ones1, d_ones), (b1s, d_b1), (b3s, d_b3),
                         (b4s, d_b4)):
                nc.sync.dma_start(t[:], d[:])
            nc.sync.dma_start(tsb[:], d_tst.rearrange("d p k -> p d k"))

            # ---- persistent activation tiles
            fbuf = bigpool.tile([128, 4 + NCH + 64], F16, tag="fbuf")
            frlin = bigpool.tile([128, NCH], F16, tag="frlin")
            specT = bigpool.tile([128, 96, BS], F16, tag="specT")
            procTs = [bigpool.tile([64, 64, BS], F16, tag=f"procT{i}",
                                   name=f"procT{i}") for i in range(C)]
            hT = bigpool.tile([128, 4, BS], F16, tag="hT")
            h1T = bigpool.tile([128, 4, BS], F16, tag="h1T")
            h2T = bigpool.tile([128, 4, BS], F16, tag="h2T")
            h_sb = bigpool.tile([32, 512], F16, tag="h_sb")
            h1_sb = bigpool.tile([32, 512], F16, tag="h1_sb")

            # ========== phase 1: fft2 (per-image m1, 8-wide m2) =============
            with tc.tile_pool(name="xpool", bufs=1) as xpool, \
                 tc.tile_pool(name="gpool", bufs=6) as gpool, \
                 tc.tile_pool(name="ps1p", bufs=4, space="PSUM") as ps1p, \
                 tc.tile_pool(name="ps2p", bufs=3, space="PSUM") as ps2p:
                xall = xpool.tile([64, NSLOT, 64], F16, tag="xall")
                for ch in range(4):
                    q0 = (NSLOT * ch) // 4
                    q1 = (NSLOT * (ch + 1)) // 4
                    nc.sync.dma_start(
                        xall[:, q0:q1, :],
                        d_x[q0:q1].rearrange("q p k -> p q k"))
                for grp in range(25):
                    n = 4 if grp < 24 else 2
                    psA = ps1p.tile([64, 512], F32, tag="psA")
                    for t in range(n):
                        i = grp * 4 + t
                        nc.tensor.matmul(psA[:, t * 128:(t + 1) * 128],
                                         xall[:, i, :], cswi[:],
                                         start=True, stop=True)
                    g1w = gpool.tile([64, 4, 128], F16, tag="g1w")
                    g1f = g1w[:, 0:n, :].rearrange("p a k -> p (a k)")
                    if grp % 2 == 0:
                        nc.scalar.copy(g1f, psA[:, 0:n * 128])
                    else:
                        nc.vector.tensor_copy(g1f, psA[:, 0:n * 128])
                    ps2 = ps2p.tile([64, 256], F32, tag="ps2")
                    nc.tensor.matmul(ps2[:, 0:n * 64], cmf[:],
                                     g1w[:, 0:n, 0:64], start=True, stop=False)
                    nc.tensor.matmul(ps2[:, 0:n * 64], msf[:],
                                     g1w[:, 0:n, 64:128], start=False, stop=True)
                    p2v = ps2.rearrange("p (k two) -> p k two", two=2)
                    if grp == 0:
                        # halo image: last 4 chunk-cols; imgs 1..3 -> cols 4:100
                        nc.vector.tensor_copy(fbuf[0:64, 0:4], p2v[:, 28:32, 0])
                        nc.vector.tensor_copy(fbuf[64:128, 0:4], p2v[:, 28:32, 1])
                        nc.vector.tensor_copy(fbuf[0:64, 4:100], p2v[:, 32:128, 0])
                        nc.vector.tensor_copy(fbuf[64:128, 4:100],
                                              p2v[:, 32:128, 1])
                    else:
                        base = 4 + (grp * 4 - 1) * 32
                        nc.vector.tensor_copy(fbuf[0:64, base:base + n * 32],
                                              p2v[:, 0:n * 32, 0])
                        nc.vector.tensor_copy(fbuf[64:128, base:base + n * 32],
                                              p2v[:, 0:n * 32, 1])

            # ================= phase 2: conv ================================
            with tc.tile_pool(name="pscv2", bufs=1, space="PSUM") as pscv2:
                psc = [pscv2.tile([128, 512], F32, tag=f"psc{i}",
                                  name=f"psc{i}") for i in range(NBLK)]
                for d in range(4):
                    for blk in range(NBLK):
                        o = 4 + blk * 512 - d
                        nc.tensor.matmul(psc[blk][:], tsb[:, d, :],
                                         fbuf[:, o:o + 512],
                                         start=(d == 0), stop=(d == 3))
                for blk in range(NBLK):
                    nc.vector.tensor_copy(frlin[:, blk * 512:(blk + 1) * 512],
                                          psc[blk][:])

            frl3 = frlin.rearrange("p (b k) -> p b k", b=BS)

            # ======= L1 / L3: acts-stationary 12288->512 + relu + transpose =
            def big_layer(src_blk, d_w, bias_row, out_sb, outT, dma_eng):
                with tc.tile_pool(name="wp", bufs=14) as wp, \
                     tc.tile_pool(name="psm", bufs=1, space="PSUM") as psm, \
                     tc.tile_pool(name="pst", bufs=1, space="PSUM") as pst:
                    acc = psm.tile([32, 512], F32, tag="acc")
                    for K4 in range(24):
                        wt = wp.tile([128, 4, 512], F16, tag="wt")
                        dma_eng.dma_start(wt[:], d_w[K4])
                        for j in range(4):
                            nc.tensor.matmul(acc[:], src_blk(4 * K4 + j),
                                             wt[:, j, :],
                                             start=(K4 == 0 and j == 0),
                                             stop=False)
                    nc.tensor.matmul(acc[:], ones1[:], bias_row[:],
                                     start=False, stop=True)
                    nc.scalar.activation(out_sb[:], acc[:], AF.Relu)
                    pt = pst.tile([128, 128], F16, tag="pt")
                    for fb in range(4):
                        nc.tensor.transpose(pt[:, fb * 32:(fb + 1) * 32],
                                            out_sb[:, fb * 128:(fb + 1) * 128],
                                            idn[:])
                    nc.vector.tensor_copy(
                        outT[:], pt.rearrange("p (f b) -> p f b", f=4))

            big_layer(lambda K: frl3[:, :, K], d_w1, b1s, h_sb, hT, nc.sync)

            # ======= L2 + L3, emission-interleaved ==========================
            # L3's k-block K only needs L2's chunk K//4, and PSUM accumulation
            # is order-independent, so L3's matmuls ride along the L2 loop.
            nc.sync.dma_start(bbig[:], d_b2[:])
            with tc.tile_pool(name="wp2", bufs=4) as wp2, \
                 tc.tile_pool(name="wp3", bufs=3) as wp3, \
                 tc.tile_pool(name="sp2", bufs=3) as sp2, \
                 tc.tile_pool(name="ps2m", bufs=3, space="PSUM") as ps2m, \
                 tc.tile_pool(name="pst2", bufs=3, space="PSUM") as pst2, \
                 tc.tile_pool(name="psm3", bufs=1, space="PSUM") as psm3:
                acc3 = psm3.tile([32, 512], F32, tag="acc3")
                for mc2 in range(12):
                    wt = wp2.tile([128, 4, 1024], F16, tag="w2")
                    nc.sync.dma_start(wt[:], d_w2[mc2])
                    for half in range(2):
                        mc = 2 * mc2 + half
                        acc = ps2m.tile([32, 512], F32, tag="acc2")
                        for fb in range(4):
                            nc.tensor.matmul(
                                acc[:], hT[:, fb, :],
                                wt[:, fb, half * 512:(half + 1) * 512],
                                start=(fb == 0), stop=False)
                        nc.tensor.matmul(acc[:], ones1[:],
                                         bbig[0:1, mc * 512:(mc + 1) * 512],
                                         start=False, stop=True)
                        sb = sp2.tile([32, 512], F16, tag="sb2")
                        if half == 0:
                            nc.scalar.copy(sb[:], acc[:])
                        else:
                            nc.vector.tensor_copy(sb[:], acc[:])
                        pt = pst2.tile([128, 128], F16, tag="pt2")
                        for fb in range(4):
                            nc.tensor.transpose(pt[:, fb * 32:(fb + 1) * 32],
                                                sb[:, fb * 128:(fb + 1) * 128],
                                                idn[:])
                        nc.vector.tensor_copy(
                            specT[:, mc * 4:(mc + 1) * 4, :],
                            pt.rearrange("p (f b) -> p f b", f=4))
                    # L3 portion: k-blocks for the two chunks just produced
                    wt3 = wp3.tile([128, 4, 512], F16, tag="wt3")
                    nc.scalar.dma_start(wt3[:], d_w3[2 * mc2])
                    wt3b = wp3.tile([128, 4, 512], F16, tag="wt3b")
                    nc.scalar.dma_start(wt3b[:], d_w3[2 * mc2 + 1])
                    for K4, w3t in ((2 * mc2, wt3), (2 * mc2 + 1, wt3b)):
                        for j in range(4):
                            nc.tensor.matmul(acc3[:],
                                             specT[:, 4 * K4 + j, :],
                                             w3t[:, j, :],
                                             start=(mc2 == 0 and K4 == 0
                                                    and j == 0),
                                             stop=False)
                nc.tensor.matmul(acc3[:], ones1[:], b3s[:],
                                 start=False, stop=True)
                nc.scalar.activation(h1_sb[:], acc3[:], AF.Relu)
                with tc.tile_pool(name="pst3", bufs=1, space="PSUM") as pst3:
                    pt = pst3.tile([128, 128], F16, tag="pt3")
                    for fb in range(4):
                        nc.tensor.transpose(pt[:, fb * 32:(fb + 1) * 32],
                                            h1_sb[:, fb * 128:(fb + 1) * 128],
                                            idn[:])
                    nc.vector.tensor_copy(
                        h1T[:], pt.rearrange("p (f b) -> p f b", f=4))

            # ======= L4: weights-stationary 512->512 + relu =================
            with tc.tile_pool(name="wp4", bufs=1) as wp4, \
                 tc.tile_pool(name="ps4m", bufs=2, space="PSUM") as ps4m:
                w4 = wp4.tile([128, 4, 512], F16, tag="w4")
                nc.gpsimd.dma_start(w4[:], d_w4.rearrange("a p k -> p a k"))
                for f2b in range(4):
                    acc = ps4m.tile([128, 32], F32, tag="acc4")
                    for fb in range(4):
                        nc.tensor.matmul(acc[:],
                                         w4[:, f2b, fb * 128:(fb + 1) * 128],
                                         h1T[:, fb, :],
                                         start=(fb == 0), stop=(fb == 3))
                    nc.scalar.activation(h2T[:, f2b, :], acc[:], AF.Relu,
                                         bias=b4s[:, f2b:f2b + 1])

            # ======= L5 + ifft2, emission-interleaved by channel ============
            nc.sync.dma_start(bbig[:], d_b5[:])
            with tc.tile_pool(name="wp5", bufs=5) as wp5, \
                 tc.tile_pool(name="sp5", bufs=3) as sp5, \
                 tc.tile_pool(name="opool", bufs=1) as opool, \
                 tc.tile_pool(name="gpi", bufs=2) as gpi, \
                 tc.tile_pool(name="ps5m", bufs=2, space="PSUM") as ps5m, \
                 tc.tile_pool(name="pst5", bufs=2, space="PSUM") as pst5, \
                 tc.tile_pool(name="ps1i", bufs=2, space="PSUM") as ps1i, \
                 tc.tile_pool(name="ps2i", bufs=2, space="PSUM") as ps2i:
                oall = opool.tile([64, NIMG * 64], F32, tag="oall")
                oal3 = oall.rearrange("u (b c v) -> u b c v", b=BS, c=C)

                def ifft2_channel(c):
                    for bg in range(BS // 4):
                        psA = ps1i.tile([64, 512], F32, tag="psAi",
                                        name="psAi")
                        for t in range(4):
                            b = bg * 4 + t
                            nc.tensor.matmul(psA[:, t * 128:(t + 1) * 128],
                                             procTs[c][:, :, b],
                                             cswi[:], start=True, stop=True)
                        g1w = gpi.tile([64, 4, 128], F16, tag="g1i",
                                       name="g1i")
                        if bg % 2 == 0:
                            nc.scalar.copy(g1w.rearrange("p a k -> p (a k)"),
                                           psA[:])
                        else:
                            nc.vector.tensor_copy(
                                g1w.rearrange("p a k -> p (a k)"), psA[:])
                        ps2 = ps2i.tile([64, 256], F32, tag="p2i", name="p2i")
                        nc.tensor.matmul(ps2[:], cmi[:], g1w[:, :, 0:64],
                                         start=True, stop=False)
                        nc.tensor.matmul(ps2[:], smi[:], g1w[:, :, 64:128],
                                         start=False, stop=True)
                        nc.scalar.copy(
                            oal3[:, bg * 4:(bg + 1) * 4, c, :],
                            ps2.rearrange("u (b v) -> u b v", b=4))
                        if c == 2:
                            for b0 in (bg * 4, bg * 4 + 2):
                                nc.sync.dma_start(
                                    d_out[b0:b0 + 2].rearrange(
                                        "b c u v -> u b c v"),
                                    oall[:, b0 * 192:(b0 + 2) * 192].rearrange(
                                        "u (b c v) -> u b c v", b=2, c=C))

                for mc2 in range(12):
                    wt = wp5.tile([128, 4, 1024], F16, tag="w5")
                    nc.gpsimd.dma_start(wt[:], d_w5[mc2])
                    for half in range(2):
                        mc = 2 * mc2 + half
                        acc = ps5m.tile([32, 512], F32, tag="acc5")
                        for fb in range(4):
                            nc.tensor.matmul(
                                acc[:], h2T[:, fb, :],
                                wt[:, fb, half * 512:(half + 1) * 512],
                                start=(fb == 0), stop=False)
                        nc.tensor.matmul(acc[:], ones1[:],
                                         bbig[0:1, mc * 512:(mc + 1) * 512],
                                         start=False, stop=True)
                        sb = sp5.tile([32, 512], F16, tag="sb5")
                        if half == 0:
                            nc.scalar.copy(sb[:], acc[:])
                        else:
                            nc.vector.tensor_copy(sb[:], acc[:])
                        pt = pst5.tile([64, 256], F16, tag="pt5")
                        for t in range(8):
                            nc.tensor.transpose(pt[:, t * 32:(t + 1) * 32],
                                                sb[:, t * 64:(t + 1) * 64],
                                                idn[:])
                        nc.vector.tensor_copy(
                            procTs[mc // 8][:, (mc % 8) * 8:(mc % 8 + 1) * 8, :],
                            pt.rearrange("p (t b) -> p t b", t=8))
                    if mc2 in (3, 7, 11):
                        ifft2_channel(mc2 // 4)

    nc.compile()
    return nc


def _get_nc():
    global _NC_CACHE
    if _NC_CACHE is None:
        _NC_CACHE = _build_nc()
    return _NC_CACHE


def _make_in_maps(x, Ws1, bs1, Ws2, bs2, Wn1, bn1, Wn2, bn2, Wn3, bn3):
    shared = dict(_host_constants())
    shared.update(_prep_weights(Ws1, bs1, Ws2, bs2, Wn1, bn1, Wn2, bn2,
                                Wn3, bn3))
    in_maps = []
    for g in range(NCORE):
        if g == 0:
            halo = np.zeros((1, 64, 64), np.float32)
        else:
            halo = x[g * BS - 1, 2][None]
        ximgs = np.concatenate(
            [halo, x[g * BS:(g + 1) * BS].reshape(NIMG, 64, 64),
             np.zeros((1, 64, 64), np.float32)]).astype(np.float16)
        in_maps.append({"ximgs": np.ascontiguousarray(ximgs), **shared})
    return in_maps


def kernel(**inputs):
    x = np.ascontiguousarray(inputs["x"], dtype=np.float32)
    nc = _get_nc()
    in_maps = _make_in_maps(
        x, inputs["Ws1"], inputs["bs1"], inputs["Ws2"], inputs["bs2"],
        inputs["Wn1"], inputs["bn1"], inputs["Wn2"], inputs["bn2"],
        inputs["Wn3"], inputs["bn3"])
    res = run_bass_kernel_spmd(nc, in_maps, list(range(NCORE)))
    out = np.empty((B, C, H, W), np.float32)
    for g in range(NCORE):
        out[g * BS:(g + 1) * BS] = res.results[g]["out"]
    return out



# revision 4
# speedup vs baseline: 3.8693x; 3.8693x over previous
"""Trainium2 Bass kernel for nn_EnhancedFractionalPINO.

Math rewrite (host-side, exact):
  * GL fractional conv is linear -> folded into Ws1:
      W1'[t,m] = sum_j w_j Ws1[t+j, m]  (causal correlation), plus a
      512x512 tail matrix for the cross-batch halo contribution.
  * spectral L2 and neural L1 have no nonlinearity between them:
      W23 = Ws2 @ Wn1 (512x512), b23 = bs2 @ Wn1 + bn1.
  * ifft2 is linear -> folded into Wn3:  G[f,:] = Re(ifft2(Wn3[f] img)).

Kernel per core (batch-parallel, 32 batches/core):
  fft2 of 96+halo images via DFT matmuls -> fbuf (flat f signal, f16)
  h0 = f @ W1' + tail @ Wtail   (W1' fp8-e3m4, per-col scales folded into
                                 the relu activation's per-partition scale)
  h1 = relu(h0 @ W23 + b23); h2 = relu(h1 @ W4 + b4)
  out = h2 @ G + gb             (G fp8-e3m4, per-row scales folded into
                                 L4's activation scale; ifft2 pre-applied)
  Everything feature-major: weights stationary (lhsT), batch=32 moving.
  Output leaves in [128, 96chunk, 32batch] layout; host transposes.
"""

import numpy as np
import ml_dtypes

import concourse.bass as bass
import concourse.mybir as mybir
import concourse.tile as tile
from concourse import bacc
from concourse.bass_utils import run_bass_kernel_spmd

F32 = mybir.dt.float32
F16 = mybir.dt.float16
F8E3 = mybir.dt.float8e3
AF = mybir.ActivationFunctionType

B, C, H, W = 256, 3, 64, 64
MODES = C * H * W              # 12288
ALPHA = 0.5
NTOT = B * MODES
NCORE = 8
BS = B // NCORE                # 32 batches per core
NIMG = BS * C                  # 96 images per core
NSLOT = NIMG + 2               # halo + 96 images + zero pad
KTAPS = 512                    # truncated GL taps
NCH = 96                       # 128-elem chunks per batch

GAM1, GAM2, GAM3 = 1.0 / 8, 0.5, 4.0
E3MAX = 14.88                  # 0.96 * e3m4 max (15.5)

W1_FP8 = True
G_FP8 = True


# ---------------------------------------------------------------- host consts
def _dft_consts():
    jk = np.outer(np.arange(64), np.arange(64)).astype(np.float64)
    Cm = np.cos(2 * np.pi * jk / 64)
    Sm = np.sin(2 * np.pi * jk / 64)
    f16 = lambda a: np.ascontiguousarray(a, dtype=np.float16)
    return {
        "cswi": f16(np.concatenate([Cm, Sm], axis=1)),     # [64, 128]
        "cmf": f16(Cm),
        "msf": f16(-Sm),
        "ones1": f16(np.ones((1, BS))),
    }


def _gl_w():
    j = np.arange(1, KTAPS, dtype=np.float64)
    return np.concatenate([[1.0], np.cumprod((j - 1.0 - ALPHA) / j)])


def _col_major(Wm, ktiles):
    """[K, M] -> [128, ktiles, M] with partition = K % 128."""
    K, M = Wm.shape
    assert K == ktiles * 128
    return np.ascontiguousarray(Wm.reshape(ktiles, 128, M).transpose(1, 0, 2))


def _pm(v):
    """[512] -> [128, 4] f32 (partition, m-tile)."""
    return np.ascontiguousarray(v.reshape(4, 128).T, dtype=np.float32)


def _prep_weights(Ws1, bs1, Ws2, bs2, Wn1, bn1, Wn2, bn2, Wn3, bn3):
    w = _gl_w()
    hscale = (1.0 / (NTOT - 1)) ** (-ALPHA)
    W1 = Ws1.astype(np.float64) * hscale

    L = 1 << 15
    wf = np.fft.rfft(w, L).conj()[:, None]
    W1p = np.fft.irfft(np.fft.rfft(W1, L, axis=0) * wf, L, axis=0)[:MODES]
    Wtail = np.zeros((KTAPS, 512))
    for p in range(1, KTAPS):
        Wtail[p] = w[KTAPS - p:] @ W1[:p]

    W23 = Ws2.astype(np.float64) @ Wn1.astype(np.float64)
    b23 = bs2.astype(np.float64) @ Wn1.astype(np.float64) + bn1
    G = np.real(np.fft.ifft2(Wn3.astype(np.float64).reshape(512, 3, 64, 64),
                             axes=(-2, -1))).reshape(512, MODES)
    gb = np.real(np.fft.ifft2(bn3.astype(np.float64).reshape(3, 64, 64),
                              axes=(-2, -1))).reshape(-1)

    if W1_FP8:
        s1 = np.abs(W1p).max(axis=0) / E3MAX
        w1q = (W1p / s1).astype(ml_dtypes.float8_e3m4)
        w1d = _col_major(w1q, 96).view(np.uint8)
    else:
        s1 = np.ones(512)
        w1d = _col_major(W1p, 96).astype(np.float16)

    if G_FP8:
        sq = np.abs(G).max(axis=1) / E3MAX
        gq = (G / sq[:, None]).astype(ml_dtypes.float8_e3m4)
        gd = np.ascontiguousarray(
            gq.reshape(4, 128, 96, 128).transpose(1, 0, 2, 3)).view(np.uint8)
    else:
        sq = np.full(512, 1.0 / 256)
        gd = np.ascontiguousarray(
            (G / sq[:, None]).reshape(4, 128, 96, 128).transpose(1, 0, 2, 3)
        ).astype(np.float16)

    f16 = lambda a: np.ascontiguousarray(a, dtype=np.float16)
    return {
        "w1q": w1d,
        "wtl": f16(_col_major(Wtail / s1[None, :], 4)),
        "w23": f16(_col_major(W23 / GAM1, 4)),
        "w4": f16(_col_major(Wn2.astype(np.float64) / GAM2, 4)),
        "gq": gd,
        "sc1": _pm(s1 * GAM1),
        "b1": _pm(bs1 * GAM1),
        "b23": _pm(b23 * GAM2),
        "sc4": _pm(GAM3 * sq),
        "b4": _pm(bn2 * GAM3 * sq),
        "gb": f16((gb * GAM3).reshape(1, MODES)),
    }


# ---------------------------------------------------------------- bass module
_NC_CACHE = {}


def _build_nc():
    nc = bacc.Bacc("TRN2", target_bir_lowering=False, debug=False,
                   num_devices=NCORE)

    def din(name, shape, dt=F16):
        return nc.dram_tensor(name, shape, dt, kind="ExternalInput")

    d_x = din("ximgs", (64, NSLOT, 64))
    d_cswi = din("cswi", (64, 128))
    d_cmf = din("cmf", (64, 64))
    d_msf = din("msf", (64, 64))
    d_ones = din("ones1", (1, BS))
    d_w1 = din("w1q", (128, 96, 512), F8E3 if W1_FP8 else F16)
    d_wtl = din("wtl", (128, 4, 512))
    d_w23 = din("w23", (128, 4, 512))
    d_w4 = din("w4", (128, 4, 512))
    d_g = din("gq", (128, 4, 96, 128), F8E3 if G_FP8 else F16)
    d_sc1 = din("sc1", (128, 4), F32)
    d_b1 = din("b1", (128, 4), F32)
    d_b23 = din("b23", (128, 4), F32)
    d_sc4 = din("sc4", (128, 4), F32)
    d_b4 = din("b4", (128, 4), F32)
    d_gb = din("gb", (1, MODES))
    d_out = nc.dram_tensor("out", (128, NCH, BS), F16, kind="ExternalOutput")

    with tile.TileContext(nc) as tc:
        with tc.tile_pool(name="cpool", bufs=1) as cpool, \
             tc.tile_pool(name="bigpool", bufs=1) as bigpool:
            cswi = cpool.tile([64, 128], F16, tag="cswi")
            cmf = cpool.tile([64, 64], F16, tag="cmf")
            msf = cpool.tile([64, 64], F16, tag="msf")
            ones1 = cpool.tile([1, BS], F16, tag="ones1")
            sc1 = cpool.tile([128, 4], F32, tag="sc1")
            b1s = cpool.tile([128, 4], F32, tag="b1s")
            b23s = cpool.tile([128, 4], F32, tag="b23s")
            sc4 = cpool.tile([128, 4], F32, tag="sc4")
            b4s = cpool.tile([128, 4], F32, tag="b4s")
            gbs = cpool.tile([1, MODES], F16, tag="gbs")
            # input + weight DMAs, priority order, all on the sync queue
            xall = bigpool.tile([64, NSLOT, 64], F16, tag="xall")
            nc.sync.dma_start(xall[:], d_x[:])
            for t, d in ((cswi, d_cswi), (cmf, d_cmf), (msf, d_msf),
                         (ones1, d_ones), (sc1, d_sc1), (b1s, d_b1),
                         (b23s, d_b23), (sc4, d_sc4), (b4s, d_b4),
                         (gbs, d_gb)):
                nc.sync.dma_start(t[:], d[:])

            w1s = bigpool.tile([128, 96, 512], F8E3 if W1_FP8 else F16,
                               tag="w1s")
            for ch in range(6):
                nc.sync.dma_start(w1s[:, 16 * ch:16 * (ch + 1), :],
                                  d_w1[:, 16 * ch:16 * (ch + 1), :])
            wtl = bigpool.tile([128, 4, 512], F16, tag="wtl")
            w23s = bigpool.tile([128, 4, 512], F16, tag="w23s")
            w4s = bigpool.tile([128, 4, 512], F16, tag="w4s")
            nc.sync.dma_start(wtl[:], d_wtl[:])
            nc.sync.dma_start(w23s[:], d_w23[:])
            nc.sync.dma_start(w4s[:], d_w4[:])
            gs = bigpool.tile([128, 4, 96, 128], F8E3 if G_FP8 else F16,
                              tag="gs")
            for ch in range(4):
                nc.sync.dma_start(gs[:, :, 24 * ch:24 * (ch + 1), :],
                                  d_g[:, :, 24 * ch:24 * (ch + 1), :])

            fbuf = bigpool.tile([128, 3140], F16, tag="fbuf")
            a1 = bigpool.tile([128, 4, BS], F16, tag="a1")
            h1 = bigpool.tile([128, 4, BS], F16, tag="h1")
            h2 = bigpool.tile([128, 4, BS], F16, tag="h2")
            stage = bigpool.tile([128, NCH, BS], F16, tag="stage")

            # ========== phase F: fft2 -> fbuf ==============================
            with tc.tile_pool(name="gpool", bufs=6) as gpool, \
                 tc.tile_pool(name="ps1p", bufs=4, space="PSUM") as ps1p, \
                 tc.tile_pool(name="ps2p", bufs=3, space="PSUM") as ps2p:
                for grp in range(25):
                    n = 4 if grp < 24 else 2
                    psA = ps1p.tile([64, 512], F32, tag="psA")
                    for t in range(n):
                        i = grp * 4 + t
                        nc.tensor.matmul(psA[:, t * 128:(t + 1) * 128],
                                         xall[:, i, :], cswi[:],
                                         start=True, stop=True)
                    g1w = gpool.tile([64, 4, 128], F16, tag="g1w")
                    g1f = g1w[:, 0:n, :].rearrange("p a k -> p (a k)")
                    if grp % 2 == 0:
                        nc.scalar.copy(g1f, psA[:, 0:n * 128])
                    else:
                        nc.vector.tensor_copy(g1f, psA[:, 0:n * 128])
                    ps2 = ps2p.tile([64, 256], F32, tag="ps2")
                    nc.tensor.matmul(ps2[:, 0:n * 64], cmf[:],
                                     g1w[:, 0:n, 0:64], start=True, stop=False)
                    nc.tensor.matmul(ps2[:, 0:n * 64], msf[:],
                                     g1w[:, 0:n, 64:128], start=False,
                                     stop=True)
                    p2v = ps2.rearrange("p (k two) -> p k two", two=2)
                    if grp == 0:
                        nc.vector.tensor_copy(fbuf[0:64, 0:4], p2v[:, 28:32, 0])
                        nc.vector.tensor_copy(fbuf[64:128, 0:4],
                                              p2v[:, 28:32, 1])
                        nc.vector.tensor_copy(fbuf[0:64, 4:100],
                                              p2v[:, 32:128, 0])
                        nc.vector.tensor_copy(fbuf[64:128, 4:100],
                                              p2v[:, 32:128, 1])
                    else:
                        base = 4 + (grp * 4 - 1) * 32
                        cp = (nc.vector.tensor_copy if grp % 2 == 0
                              else nc.scalar.copy)
                        cp(fbuf[0:64, base:base + n * 32], p2v[:, 0:n * 32, 0])
                        cp(fbuf[64:128, base:base + n * 32],
                           p2v[:, 0:n * 32, 1])

            fview = fbuf[:, 4:4 + BS * 96].rearrange("p (b k) -> p b k", b=BS)
            ftail = fbuf[:, 0:BS * 96].rearrange("p (b k) -> p b k", b=BS)

            # ========== L1: h0 = f @ W1' + tail, relu ======================
            with tc.tile_pool(name="ps1m", bufs=1, space="PSUM") as ps1m:
                psL = [ps1m.tile([128, BS], F32, tag=f"psL{m}",
                                 name=f"psL{m}") for m in range(4)]
                for j in range(96):
                    for m in range(4):
                        nc.tensor.matmul(psL[m][:],
                                         w1s[:, j, m * 128:(m + 1) * 128],
                                         fview[:, :, j],
                                         start=(j == 0), stop=False)
                for jt in range(4):
                    for m in range(4):
                        nc.tensor.matmul(psL[m][:],
                                         wtl[:, jt, m * 128:(m + 1) * 128],
                                         ftail[:, :, jt],
                                         start=False, stop=(jt == 3))
                for m in range(4):
                    nc.scalar.activation(a1[:, m, :], psL[m][:], AF.Relu,
                                         bias=b1s[:, m:m + 1],
                                         scale=sc1[:, m:m + 1])

            # ========== L23 / L4: 512x512 layers ===========================
            with tc.tile_pool(name="ps2m", bufs=4, space="PSUM") as ps2m:
                for m in range(4):
                    acc = ps2m.tile([128, BS], F32, tag="acc23")
                    for k in range(4):
                        nc.tensor.matmul(acc[:],
                                         w23s[:, k, m * 128:(m + 1) * 128],
                                         a1[:, k, :],
                                         start=(k == 0), stop=(k == 3))
                    nc.scalar.activation(h1[:, m, :], acc[:], AF.Relu,
                                         bias=b23s[:, m:m + 1], scale=GAM2)
                for m in range(4):
                    acc = ps2m.tile([128, BS], F32, tag="acc4")
                    for k in range(4):
                        nc.tensor.matmul(acc[:],
                                         w4s[:, k, m * 128:(m + 1) * 128],
                                         h1[:, k, :],
                                         start=(k == 0), stop=(k == 3))
                    nc.scalar.activation(h2[:, m, :], acc[:], AF.Relu,
                                         bias=b4s[:, m:m + 1],
                                         scale=sc4[:, m:m + 1])

            # ========== L5: out = h2 @ G + gb (ifft2 pre-folded) ===========
            with tc.tile_pool(name="ps5m", bufs=8, space="PSUM") as ps5m:
                for c in range(NCH):
                    acc = ps5m.tile([128, BS], F32, tag="acc5")
                    for k in range(4):
                        nc.tensor.matmul(acc[:], gs[:, k, c, :], h2[:, k, :],
                                         start=(k == 0), stop=False)
                    nc.tensor.matmul(acc[:],
                                     gbs[0:1, c * 128:(c + 1) * 128],
                                     ones1[:], start=False, stop=True)
                    if c % 2 == 0:
                        nc.scalar.copy(stage[:, c, :], acc[:])
                    else:
                        nc.vector.tensor_copy(stage[:, c, :], acc[:])
                    if c % 8 == 7:
                        c0 = c - 7
                        nc.sync.dma_start(d_out[:, c0:c0 + 8, :],
                                          stage[:, c0:c0 + 8, :])

    nc.compile()
    return nc


def _get_nc():
    key = (W1_FP8, G_FP8)
    if key not in _NC_CACHE:
        _NC_CACHE[key] = _build_nc()
    return _NC_CACHE[key]


def _make_in_maps(x, Ws1, bs1, Ws2, bs2, Wn1, bn1, Wn2, bn2, Wn3, bn3):
    shared = dict(_dft_consts())
    shared.update(_prep_weights(Ws1, bs1, Ws2, bs2, Wn1, bn1, Wn2, bn2,
                                Wn3, bn3))
    in_maps = []
    for g in range(NCORE):
        if g == 0:
            halo = np.zeros((1, 64, 64), np.float32)
        else:
            halo = x[g * BS - 1, 2][None]
        ximgs = np.concatenate(
            [halo, x[g * BS:(g + 1) * BS].reshape(NIMG, 64, 64),
             np.zeros((1, 64, 64), np.float32)]).astype(np.float16)
        in_maps.append({"ximgs": np.ascontiguousarray(
            ximgs.transpose(1, 0, 2)), **shared})
    return in_maps


def kernel(**inputs):
    x = np.ascontiguousarray(inputs["x"], dtype=np.float32)
    nc = _get_nc()
    in_maps = _make_in_maps(
        x, inputs["Ws1"], inputs["bs1"], inputs["Ws2"], inputs["bs2"],
        inputs["Wn1"], inputs["bn1"], inputs["Wn2"], inputs["bn2"],
        inputs["Wn3"], inputs["bn3"])
    res = run_bass_kernel_spmd(nc, in_maps, list(range(NCORE)))
    out = np.empty((B, C, H, W), np.float32)
    for g in range(NCORE):
        st = np.asarray(res.results[g]["out"], dtype=np.float32) / GAM3
        out[g * BS:(g + 1) * BS] = st.transpose(2, 1, 0).reshape(
            BS, C, H, W)
    return out


# revision 13
# speedup vs baseline: 4.3055x; 1.1127x over previous
"""Trainium2 Bass kernel for nn_EnhancedFractionalPINO.

Math rewrite (host-side, exact):
  * GL fractional conv is linear -> folded into Ws1:
      W1'[t,m] = sum_j w_j Ws1[t+j, m]  (causal correlation), plus a
      512x512 tail matrix for the cross-batch halo contribution.
  * spectral L2 and neural L1 have no nonlinearity between them:
      W23 = Ws2 @ Wn1 (512x512), b23 = bs2 @ Wn1 + bn1.
  * ifft2 is linear -> folded into Wn3:  G[f,:] = Re(ifft2(Wn3[f] img)).

Kernel per core (batch-parallel, 32 batches/core):
  fft2 of 96+halo images via DFT matmuls -> fbuf (flat f signal, f16)
  h0 = f @ W1' + tail @ Wtail   (W1' fp8-e3m4, per-col scales folded into
                                 the relu activation's per-partition scale)
  h1 = relu(h0 @ W23 + b23); h2 = relu(h1 @ W4 + b4)
  out = h2 @ G + gb             (G fp8-e3m4, per-row scales folded into
                                 L4's activation scale; ifft2 pre-applied)
  Everything feature-major: weights stationary (lhsT), batch=32 moving.
  Output leaves in [128, 96chunk, 32batch] layout; host transposes.
"""

import numpy as np
import ml_dtypes

import concourse.bass as bass
import concourse.mybir as mybir
import concourse.tile as tile
from concourse import bacc
from concourse.bass_utils import run_bass_kernel_spmd

F32 = mybir.dt.float32
F16 = mybir.dt.float16
F8E3 = mybir.dt.float8e3
AF = mybir.ActivationFunctionType

B, C, H, W = 256, 3, 64, 64
MODES = C * H * W              # 12288
ALPHA = 0.5
NTOT = B * MODES
NCORE = 8
BS = B // NCORE                # 32 batches per core
NIMG = BS * C                  # 96 images per core
NSLOT = NIMG + 2               # halo + 96 images + zero pad
KTAPS = 512                    # truncated GL taps
NCH = 96                       # 128-elem chunks per batch

GAM1, GAM2, GAM3 = 1.0 / 8, 0.5, 4.0
E3MAX = 14.88                  # 0.96 * e3m4 max (15.5)

W1_FP8 = True
G_FP8 = True


# ---------------------------------------------------------------- host consts
def _dft_consts():
    jk = np.outer(np.arange(64), np.arange(64)).astype(np.float64)
    Cm = np.cos(2 * np.pi * jk / 64)
    Sm = np.sin(2 * np.pi * jk / 64)
    # [cswi | cmf | msf | ones(row0)] packed into one DMA
    pack = np.zeros((64, 256 + BS))
    pack[:, 0:64] = Cm
    pack[:, 64:128] = Sm
    pack[:, 128:192] = Cm
    pack[:, 192:256] = -Sm
    pack[0, 256:256 + BS] = 1.0
    return {"fftpk": np.ascontiguousarray(pack, dtype=np.float16)}


def _gl_w():
    j = np.arange(1, KTAPS, dtype=np.float64)
    return np.concatenate([[1.0], np.cumprod((j - 1.0 - ALPHA) / j)])


def _col_major(Wm, ktiles):
    """[K, M] -> [128, ktiles, M] with partition = K % 128."""
    K, M = Wm.shape
    assert K == ktiles * 128
    return np.ascontiguousarray(Wm.reshape(ktiles, 128, M).transpose(1, 0, 2))


def _pm(v):
    """[512] -> [128, 4] f32 (partition, m-tile)."""
    return np.ascontiguousarray(v.reshape(4, 128).T, dtype=np.float32)


def _prep_weights(Ws1, bs1, Ws2, bs2, Wn1, bn1, Wn2, bn2, Wn3, bn3):
    w = _gl_w()
    hscale = (1.0 / (NTOT - 1)) ** (-ALPHA)
    W1 = Ws1.astype(np.float64) * hscale

    L = 1 << 15
    wf = np.fft.rfft(w, L).conj()[:, None]
    W1p = np.fft.irfft(np.fft.rfft(W1, L, axis=0) * wf, L, axis=0)[:MODES]
    Wtail = np.zeros((KTAPS, 512))
    for p in range(1, KTAPS):
        Wtail[p] = w[KTAPS - p:] @ W1[:p]

    W23 = Ws2.astype(np.float64) @ Wn1.astype(np.float64)
    b23 = bs2.astype(np.float64) @ Wn1.astype(np.float64) + bn1
    G = np.real(np.fft.ifft2(Wn3.astype(np.float64).reshape(512, 3, 64, 64),
                             axes=(-2, -1))).reshape(512, MODES)
    gb = np.real(np.fft.ifft2(bn3.astype(np.float64).reshape(3, 64, 64),
                              axes=(-2, -1))).reshape(-1)

    if W1_FP8:
        s1 = np.abs(W1p).max(axis=0) / E3MAX
        w1q = (W1p / s1).astype(ml_dtypes.float8_e3m4)
        w1d = _col_major(w1q, 96).view(np.uint8)
    else:
        s1 = np.ones(512)
        w1d = _col_major(W1p, 96).astype(np.float16)

    if G_FP8:
        sq = np.abs(G).max(axis=1) / E3MAX
        gq = (G / sq[:, None]).astype(ml_dtypes.float8_e3m4)
        gd = np.ascontiguousarray(
            gq.reshape(4, 128, 96, 128).transpose(1, 0, 2, 3)).view(np.uint8)
    else:
        sq = np.full(512, 1.0 / 256)
        gd = np.ascontiguousarray(
            (G / sq[:, None]).reshape(4, 128, 96, 128).transpose(1, 0, 2, 3)
        ).astype(np.float16)

    f16 = lambda a: np.ascontiguousarray(a, dtype=np.float16)
    wpack = np.concatenate([_col_major(W23 / GAM1, 4),
                            _col_major(Wn2.astype(np.float64) / GAM2, 4)],
                           axis=1)                          # [128, 8, 512]
    spack = np.concatenate([_pm(s1 * GAM1), _pm(bs1 * GAM1),
                            _pm(b23 * GAM2), _pm(GAM3 * sq),
                            _pm(bn2 * GAM3 * sq)], axis=1)  # [128, 20]
    return {
        "w1q": w1d,
        "wtl8": _col_major(
            (Wtail / s1[None, :]).astype(ml_dtypes.float8_e3m4), 4
        ).view(np.uint8),
        "wpk": f16(wpack),
        "gq": gd,
        "spk": np.ascontiguousarray(spack, dtype=np.float32),
        "gb": f16((gb * GAM3).reshape(1, MODES)),
    }


# ---------------------------------------------------------------- bass module
_NC_CACHE = {}


def _build_nc():
    nc = bacc.Bacc("TRN2", target_bir_lowering=False, debug=False,
                   num_devices=NCORE)

    def din(name, shape, dt=F16):
        return nc.dram_tensor(name, shape, dt, kind="ExternalInput")

    d_x = din("ximgs", (64, NSLOT, 64))
    d_fpk = din("fftpk", (64, 256 + BS))
    d_w1 = din("w1q", (128, 96, 512), F8E3 if W1_FP8 else F16)
    d_wtl = din("wtl8", (128, 4, 512), F8E3)
    d_wpk = din("wpk", (128, 8, 512))
    d_g = din("gq", (128, 4, 96, 128), F8E3 if G_FP8 else F16)
    d_spk = din("spk", (128, 20), F32)
    d_gb = din("gb", (1, MODES))
    d_out = nc.dram_tensor("out", (128, NCH, BS), F16, kind="ExternalOutput")

    with tile.TileContext(nc) as tc:
        with tc.tile_pool(name="cpool", bufs=1) as cpool, \
             tc.tile_pool(name="bigpool", bufs=1) as bigpool:
            fpk = cpool.tile([64, 256 + BS], F16, tag="fpk")
            spk = cpool.tile([128, 20], F32, tag="spk")
            gbs = cpool.tile([1, MODES], F16, tag="gbs")
            cswi, cmf, msf = fpk[:, 0:128], fpk[:, 128:192], fpk[:, 192:256]
            ones1 = fpk[0:1, 256:256 + BS]
            sc1, b1s = spk[:, 0:4], spk[:, 4:8]
            b23s, sc4, b4s = spk[:, 8:12], spk[:, 12:16], spk[:, 16:20]
            # big DMAs in priority order on the sync queue; small packs on
            # the scalar queue so they don't hold up the stream
            xall = bigpool.tile([64, NSLOT, 64], F16, tag="xall")
            nc.sync.dma_start(xall[:], d_x[:])
            nc.scalar.dma_start(fpk[:], d_fpk[:])
            nc.scalar.dma_start(spk[:], d_spk[:])
            nc.scalar.dma_start(gbs[:], d_gb[:])

            w1s = bigpool.tile([128, 96, 512], F8E3 if W1_FP8 else F16,
                               tag="w1s")
            for ch in range(6):
                nc.sync.dma_start(w1s[:, 16 * ch:16 * (ch + 1), :],
                                  d_w1[:, 16 * ch:16 * (ch + 1), :])
            wtl = bigpool.tile([128, 4, 512], F8E3, tag="wtl")
            nc.sync.dma_start(wtl[:], d_wtl[:])
            wpk = bigpool.tile([128, 8, 512], F16, tag="wpk")
            nc.sync.dma_start(wpk[:], d_wpk[:])
            w23s = wpk[:, 0:4, :]
            w4s = wpk[:, 4:8, :]
            gs = bigpool.tile([128, 4, 96, 128], F8E3 if G_FP8 else F16,
                              tag="gs")
            for ch in range(12):
                nc.sync.dma_start(gs[:, :, 8 * ch:8 * (ch + 1), :],
                                  d_g[:, :, 8 * ch:8 * (ch + 1), :])

            fbuf = bigpool.tile([128, 3140], F16, tag="fbuf")
            a1 = bigpool.tile([128, 4, BS], F16, tag="a1")
            h1 = bigpool.tile([128, 4, BS], F16, tag="h1")
            h2 = bigpool.tile([128, 4, BS], F16, tag="h2")
            stage = bigpool.tile([128, NCH, BS], F16, tag="stage")

            # ========== phase F: fft2 -> fbuf ==============================
            with tc.tile_pool(name="gpool", bufs=6) as gpool, \
                 tc.tile_pool(name="ps1p", bufs=4, space="PSUM") as ps1p, \
                 tc.tile_pool(name="ps2p", bufs=3, space="PSUM") as ps2p:
                for grp in range(25):
                    n = 4 if grp < 24 else 2
                    psA = ps1p.tile([64, 512], F32, tag="psA")
                    for t in range(n):
                        i = grp * 4 + t
                        nc.tensor.matmul(psA[:, t * 128:(t + 1) * 128],
                                         xall[:, i, :], cswi,
                                         start=True, stop=True)
                    g1w = gpool.tile([64, 4, 128], F16, tag="g1w")
                    g1f = g1w[:, 0:n, :].rearrange("p a k -> p (a k)")
                    if grp % 2 == 0:
                        nc.scalar.copy(g1f, psA[:, 0:n * 128])
                    else:
                        nc.vector.tensor_copy(g1f, psA[:, 0:n * 128])
                    ps2 = ps2p.tile([64, 256], F32, tag="ps2")
                    nc.tensor.matmul(ps2[:, 0:n * 64], cmf,
                                     g1w[:, 0:n, 0:64], start=True, stop=False)
                    nc.tensor.matmul(ps2[:, 0:n * 64], msf,
                                     g1w[:, 0:n, 64:128], start=False,
                                     stop=True)
                    p2v = ps2.rearrange("p (k two) -> p k two", two=2)
                    if grp == 0:
                        nc.vector.tensor_copy(fbuf[0:64, 0:4], p2v[:, 28:32, 0])
                        nc.vector.tensor_copy(fbuf[64:128, 0:4],
                                              p2v[:, 28:32, 1])
                        nc.vector.tensor_copy(fbuf[0:64, 4:100],
                                              p2v[:, 32:128, 0])
                        nc.vector.tensor_copy(fbuf[64:128, 4:100],
                                              p2v[:, 32:128, 1])
                    else:
                        base = 4 + (grp * 4 - 1) * 32
                        cp = (nc.vector.tensor_copy if grp % 2 == 0
                              else nc.scalar.copy)
                        cp(fbuf[0:64, base:base + n * 32], p2v[:, 0:n * 32, 0])
                        cp(fbuf[64:128, base:base + n * 32],
                           p2v[:, 0:n * 32, 1])

            fview = fbuf[:, 4:4 + BS * 96].rearrange("p (b k) -> p b k", b=BS)
            ftail = fbuf[:, 0:BS * 96].rearrange("p (b k) -> p b k", b=BS)

            # ========== L1: h0 = f @ W1' + tail, relu ======================
            with tc.tile_pool(name="ps1m", bufs=1, space="PSUM") as ps1m:
                psL = [ps1m.tile([128, BS], F32, tag=f"psL{m}",
                                 name=f"psL{m}") for m in range(4)]
                for j in range(96):
                    for m in range(4):
                        nc.tensor.matmul(psL[m][:],
                                         w1s[:, j, m * 128:(m + 1) * 128],
                                         fview[:, :, j],
                                         start=(j == 0), stop=False)
                for jt in range(4):
                    for m in range(4):
                        nc.tensor.matmul(psL[m][:],
                                         wtl[:, jt, m * 128:(m + 1) * 128],
                                         ftail[:, :, jt],
                                         start=False, stop=(jt == 3))
                for m in range(4):
                    nc.scalar.activation(a1[:, m, :], psL[m][:], AF.Relu,
                                         bias=b1s[:, m:m + 1],
                                         scale=sc1[:, m:m + 1])

            # ========== L23 / L4: 512x512 layers ===========================
            with tc.tile_pool(name="ps2m", bufs=4, space="PSUM") as ps2m:
                for m in range(4):
                    acc = ps2m.tile([128, BS], F32, tag="acc23")
                    for k in range(4):
                        nc.tensor.matmul(acc[:],
                                         w23s[:, k, m * 128:(m + 1) * 128],
                                         a1[:, k, :],
                                         start=(k == 0), stop=(k == 3))
                    nc.scalar.activation(h1[:, m, :], acc[:], AF.Relu,
                                         bias=b23s[:, m:m + 1], scale=GAM2)
                for m in range(4):
                    acc = ps2m.tile([128, BS], F32, tag="acc4")
                    for k in range(4):
                        nc.tensor.matmul(acc[:],
                                         w4s[:, k, m * 128:(m + 1) * 128],
                                         h1[:, k, :],
                                         start=(k == 0), stop=(k == 3))
                    nc.scalar.activation(h2[:, m, :], acc[:], AF.Relu,
                                         bias=b4s[:, m:m + 1],
                                         scale=sc4[:, m:m + 1])

            # ========== L5: out = h2 @ G + gb (ifft2 pre-folded) ===========
            with tc.tile_pool(name="ps5m", bufs=8, space="PSUM") as ps5m:
                for c in range(NCH):
                    acc = ps5m.tile([128, BS], F32, tag="acc5")
                    for k in range(4):
                        nc.tensor.matmul(acc[:], gs[:, k, c, :], h2[:, k, :],
                                         start=(k == 0), stop=False)
                    nc.tensor.matmul(acc[:],
                                     gbs[0:1, c * 128:(c + 1) * 128],
                                     ones1, start=False, stop=True)
                    if c % 2 == 0:
                        nc.scalar.copy(stage[:, c, :], acc[:])
                    else:
                        nc.vector.tensor_copy(stage[:, c, :], acc[:])
                    if c % 8 == 7:
                        c0 = c - 7
                        nc.sync.dma_start(d_out[:, c0:c0 + 8, :],
                                            stage[:, c0:c0 + 8, :])

    nc.compile()
    return nc


def _get_nc():
    key = (W1_FP8, G_FP8)
    if key not in _NC_CACHE:
        _NC_CACHE[key] = _build_nc()
    return _NC_CACHE[key]


def _make_in_maps(x, Ws1, bs1, Ws2, bs2, Wn1, bn1, Wn2, bn2, Wn3, bn3):
    shared = dict(_dft_consts())
    shared.update(_prep_weights(Ws1, bs1, Ws2, bs2, Wn1, bn1, Wn2, bn2,
                                Wn3, bn3))
    in_maps = []
    for g in range(NCORE):
        if g == 0:
            halo = np.zeros((1, 64, 64), np.float32)
        else:
            halo = x[g * BS - 1, 2][None]
        ximgs = np.concatenate(
            [halo, x[g * BS:(g + 1) * BS].reshape(NIMG, 64, 64),
             np.zeros((1, 64, 64), np.float32)]).astype(np.float16)
        in_maps.append({"ximgs": np.ascontiguousarray(
            ximgs.transpose(1, 0, 2)), **shared})
    return in_maps


def kernel(**inputs):
    x = np.ascontiguousarray(inputs["x"], dtype=np.float32)
    nc = _get_nc()
    in_maps = _make_in_maps(
        x, inputs["Ws1"], inputs["bs1"], inputs["Ws2"], inputs["bs2"],
        inputs["Wn1"], inputs["bn1"], inputs["Wn2"], inputs["bn2"],
        inputs["Wn3"], inputs["bn3"])
    res = run_bass_kernel_spmd(nc, in_maps, list(range(NCORE)))
    out = np.empty((B, C, H, W), np.float32)
    for g in range(NCORE):
        st = np.asarray(res.results[g]["out"], dtype=np.float32) / GAM3
        out[g * BS:(g + 1) * BS] = st.transpose(2, 1, 0).reshape(
            BS, C, H, W)
    return out


# revision 17
# speedup vs baseline: 4.4875x; 1.0423x over previous
"""Trainium2 Bass kernel for nn_EnhancedFractionalPINO.

Math rewrite (host-side, exact):
  * GL fractional conv is linear -> folded into Ws1:
      W1'[t,m] = sum_j w_j Ws1[t+j, m]  (causal correlation), plus a
      512x512 tail matrix for the cross-batch halo contribution.
  * spectral L2 and neural L1 have no nonlinearity between them:
      W23 = Ws2 @ Wn1 (512x512), b23 = bs2 @ Wn1 + bn1.
  * ifft2 is linear -> folded into Wn3:  G[f,:] = Re(ifft2(Wn3[f] img)).

Kernel per core (batch-parallel, 32 batches/core):
  fft2 of 96+halo images via DFT matmuls -> fbuf (flat f signal, f16)
  h0 = f @ W1' + tail @ Wtail   (W1' fp8-e3m4, per-col scales folded into
                                 the relu activation's per-partition scale)
  h1 = relu(h0 @ W23 + b23); h2 = relu(h1 @ W4 + b4)
  out = h2 @ G + gb             (G fp8-e3m4, per-row scales folded into
                                 L4's activation scale; ifft2 pre-applied)
  Everything feature-major: weights stationary (lhsT), batch=32 moving.
  Output leaves in [128, 96chunk, 32batch] layout; host transposes.
"""

import numpy as np
import ml_dtypes

import concourse.bass as bass
import concourse.mybir as mybir
import concourse.tile as tile
from concourse import bacc
from concourse.bass_utils import run_bass_kernel_spmd

F32 = mybir.dt.float32
F16 = mybir.dt.float16
F8E3 = mybir.dt.float8e3
AF = mybir.ActivationFunctionType

B, C, H, W = 256, 3, 64, 64
MODES = C * H * W              # 12288
ALPHA = 0.5
NTOT = B * MODES
NCORE = 8
BS = B // NCORE                # 32 batches per core
NIMG = BS * C                  # 96 images per core
NSLOT = NIMG + 2               # halo + 96 images + zero pad
KTAPS = 512                    # truncated GL taps
NCH = 96                       # 128-elem chunks per batch

GAM1, GAM2, GAM3 = 1.0 / 8, 0.5, 4.0
E3MAX = 14.88                  # 0.96 * e3m4 max (15.5)

W1_FP8 = True
G_FP8 = True


# ---------------------------------------------------------------- host consts
def _dft_consts():
    jk = np.outer(np.arange(64), np.arange(64)).astype(np.float64)
    Cm = np.cos(2 * np.pi * jk / 64)
    Sm = np.sin(2 * np.pi * jk / 64)
    # [cswi | cmf | msf | ones(row0)] packed into one DMA
    pack = np.zeros((64, 256 + BS))
    pack[:, 0:64] = Cm
    pack[:, 64:128] = Sm
    pack[:, 128:192] = Cm
    pack[:, 192:256] = -Sm
    pack[0, 256:256 + BS] = 1.0
    return {"fftpk": np.ascontiguousarray(pack, dtype=np.float16)}


def _gl_w():
    j = np.arange(1, KTAPS, dtype=np.float64)
    return np.concatenate([[1.0], np.cumprod((j - 1.0 - ALPHA) / j)])


def _col_major(Wm, ktiles):
    """[K, M] -> [128, ktiles, M] with partition = K % 128."""
    K, M = Wm.shape
    assert K == ktiles * 128
    return np.ascontiguousarray(Wm.reshape(ktiles, 128, M).transpose(1, 0, 2))


def _pm(v):
    """[512] -> [128, 4] f32 (partition, m-tile)."""
    return np.ascontiguousarray(v.reshape(4, 128).T, dtype=np.float32)


def _prep_weights(Ws1, bs1, Ws2, bs2, Wn1, bn1, Wn2, bn2, Wn3, bn3):
    w = _gl_w()
    hscale = (1.0 / (NTOT - 1)) ** (-ALPHA)
    W1 = Ws1.astype(np.float64) * hscale

    L = 1 << 15
    wf = np.fft.rfft(w, L).conj()[:, None]
    W1p = np.fft.irfft(np.fft.rfft(W1, L, axis=0) * wf, L, axis=0)[:MODES]
    Wtail = np.zeros((KTAPS, 512))
    for p in range(1, KTAPS):
        Wtail[p] = w[KTAPS - p:] @ W1[:p]

    W23 = Ws2.astype(np.float64) @ Wn1.astype(np.float64)
    b23 = bs2.astype(np.float64) @ Wn1.astype(np.float64) + bn1
    G = np.real(np.fft.ifft2(Wn3.astype(np.float64).reshape(512, 3, 64, 64),
                             axes=(-2, -1))).reshape(512, MODES)
    gb = np.real(np.fft.ifft2(bn3.astype(np.float64).reshape(3, 64, 64),
                              axes=(-2, -1))).reshape(-1)

    if W1_FP8:
        s1 = np.abs(W1p).max(axis=0) / E3MAX
        w1q = (W1p / s1).astype(ml_dtypes.float8_e3m4)
        w1d = _col_major(w1q, 96).view(np.uint8)
    else:
        s1 = np.ones(512)
        w1d = _col_major(W1p, 96).astype(np.float16)

    if G_FP8:
        sq = np.abs(G).max(axis=1) / E3MAX
        gq = (G / sq[:, None]).astype(ml_dtypes.float8_e3m4)
        gd = np.ascontiguousarray(
            gq.reshape(4, 128, 96, 128).transpose(1, 0, 2, 3)).view(np.uint8)
    else:
        sq = np.full(512, 1.0 / 256)
        gd = np.ascontiguousarray(
            (G / sq[:, None]).reshape(4, 128, 96, 128).transpose(1, 0, 2, 3)
        ).astype(np.float16)

    f16 = lambda a: np.ascontiguousarray(a, dtype=np.float16)
    wpack = np.concatenate([_col_major(W23 / GAM1, 4),
                            _col_major(Wn2.astype(np.float64) / GAM2, 4)],
                           axis=1)                          # [128, 8, 512]
    spack = np.concatenate([_pm(s1 * GAM1), _pm(bs1 * GAM1),
                            _pm(b23 * GAM2), _pm(GAM3 * sq),
                            _pm(bn2 * GAM3 * sq)], axis=1)  # [128, 20]
    return {
        "w1q": w1d,
        "wtl8": _col_major(
            (Wtail / s1[None, :]).astype(ml_dtypes.float8_e3m4), 4
        ).view(np.uint8),
        "wpk": f16(wpack),
        "gq": gd,
        "spk": np.ascontiguousarray(spack, dtype=np.float32),
        "gb": f16((gb * GAM3).reshape(1, MODES)),
    }


# ---------------------------------------------------------------- bass module
_NC_CACHE = {}


def _build_nc():
    nc = bacc.Bacc("TRN2", target_bir_lowering=False, debug=False,
                   num_devices=NCORE)

    def din(name, shape, dt=F16):
        return nc.dram_tensor(name, shape, dt, kind="ExternalInput")

    d_x = din("ximgs", (64, NSLOT, 64))
    d_fpk = din("fftpk", (64, 256 + BS))
    d_w1 = din("w1q", (128, 96, 512), F8E3 if W1_FP8 else F16)
    d_wtl = din("wtl8", (128, 4, 512), F8E3)
    d_wpk = din("wpk", (128, 8, 512))
    d_g = din("gq", (128, 4, 96, 128), F8E3 if G_FP8 else F16)
    d_spk = din("spk", (128, 20), F32)
    d_gb = din("gb", (1, MODES))
    d_out = nc.dram_tensor("out", (128, NCH, BS), F16, kind="ExternalOutput")

    with tile.TileContext(nc) as tc:
        with tc.tile_pool(name="cpool", bufs=1) as cpool, \
             tc.tile_pool(name="bigpool", bufs=1) as bigpool:
            fpk = cpool.tile([64, 256 + BS], F16, tag="fpk")
            spk = cpool.tile([128, 20], F32, tag="spk")
            gbs = cpool.tile([1, MODES], F16, tag="gbs")
            cswi, cmf, msf = fpk[:, 0:128], fpk[:, 128:192], fpk[:, 192:256]
            ones1 = fpk[0:1, 256:256 + BS]
            sc1, b1s = spk[:, 0:4], spk[:, 4:8]
            b23s, sc4, b4s = spk[:, 8:12], spk[:, 12:16], spk[:, 16:20]
            # big DMAs in priority order on the sync queue; small packs on
            # the scalar queue so they don't hold up the stream
            xall = bigpool.tile([64, NSLOT, 64], F16, tag="xall")
            nc.sync.dma_start(xall[:], d_x[:])
            nc.scalar.dma_start(fpk[:], d_fpk[:])
            nc.scalar.dma_start(spk[:], d_spk[:])
            nc.scalar.dma_start(gbs[:], d_gb[:])

            w1s = bigpool.tile([128, 96, 512], F8E3 if W1_FP8 else F16,
                               tag="w1s")
            for ch in range(6):
                nc.sync.dma_start(w1s[:, 16 * ch:16 * (ch + 1), :],
                                  d_w1[:, 16 * ch:16 * (ch + 1), :])
            wtl = bigpool.tile([128, 4, 512], F8E3, tag="wtl")
            nc.sync.dma_start(wtl[:], d_wtl[:])
            wpk = bigpool.tile([128, 8, 512], F16, tag="wpk")
            nc.sync.dma_start(wpk[:], d_wpk[:])
            w23s = wpk[:, 0:4, :]
            w4s = wpk[:, 4:8, :]
            gs = bigpool.tile([128, 4, 96, 128], F8E3 if G_FP8 else F16,
                              tag="gs")
            gchunks = [(8 * i, 8 * (i + 1)) for i in range(10)] + \
                      [(80 + 4 * i, 84 + 4 * i) for i in range(4)]
            for c0, c1 in gchunks:
                nc.sync.dma_start(gs[:, :, c0:c1, :], d_g[:, :, c0:c1, :])

            fbuf = bigpool.tile([128, 3140], F16, tag="fbuf")
            a1 = bigpool.tile([128, 4, BS], F16, tag="a1")
            h1 = bigpool.tile([128, 4, BS], F16, tag="h1")
            h2 = bigpool.tile([128, 4, BS], F16, tag="h2")
            stage = bigpool.tile([128, NCH, BS], F16, tag="stage")

            # ========== phase F: fft2 -> fbuf ==============================
            with tc.tile_pool(name="gpool", bufs=6) as gpool, \
                 tc.tile_pool(name="ps1p", bufs=4, space="PSUM") as ps1p, \
                 tc.tile_pool(name="ps2p", bufs=3, space="PSUM") as ps2p:
                for grp in range(25):
                    n = 4 if grp < 24 else 2
                    psA = ps1p.tile([64, 512], F32, tag="psA")
                    for t in range(n):
                        i = grp * 4 + t
                        nc.tensor.matmul(psA[:, t * 128:(t + 1) * 128],
                                         xall[:, i, :], cswi,
                                         start=True, stop=True)
                    g1w = gpool.tile([64, 4, 128], F16, tag="g1w")
                    g1f = g1w[:, 0:n, :].rearrange("p a k -> p (a k)")
                    if grp % 2 == 0:
                        nc.scalar.copy(g1f, psA[:, 0:n * 128])
                    else:
                        nc.vector.tensor_copy(g1f, psA[:, 0:n * 128])
                    ps2 = ps2p.tile([64, 256], F32, tag="ps2")
                    nc.tensor.matmul(ps2[:, 0:n * 64], cmf,
                                     g1w[:, 0:n, 0:64], start=True, stop=False)
                    nc.tensor.matmul(ps2[:, 0:n * 64], msf,
                                     g1w[:, 0:n, 64:128], start=False,
                                     stop=True)
                    p2v = ps2.rearrange("p (k two) -> p k two", two=2)
                    if grp == 0:
                        nc.vector.tensor_copy(fbuf[0:64, 0:4], p2v[:, 28:32, 0])
                        nc.vector.tensor_copy(fbuf[64:128, 0:4],
                                              p2v[:, 28:32, 1])
                        nc.vector.tensor_copy(fbuf[0:64, 4:100],
                                              p2v[:, 32:128, 0])
                        nc.vector.tensor_copy(fbuf[64:128, 4:100],
                                              p2v[:, 32:128, 1])
                    else:
                        base = 4 + (grp * 4 - 1) * 32
                        cp = (nc.vector.tensor_copy if grp % 2 == 0
                              else nc.scalar.copy)
                        cp(fbuf[0:64, base:base + n * 32], p2v[:, 0:n * 32, 0])
                        cp(fbuf[64:128, base:base + n * 32],
                           p2v[:, 0:n * 32, 1])

            fview = fbuf[:, 4:4 + BS * 96].rearrange("p (b k) -> p b k", b=BS)
            ftail = fbuf[:, 0:BS * 96].rearrange("p (b k) -> p b k", b=BS)

            # ========== L1: h0 = f @ W1' + tail, relu ======================
            with tc.tile_pool(name="ps1m", bufs=1, space="PSUM") as ps1m:
                psL = [ps1m.tile([128, BS], F32, tag=f"psL{m}",
                                 name=f"psL{m}") for m in range(4)]
                for j in range(96):
                    for m in range(4):
                        nc.tensor.matmul(psL[m][:],
                                         w1s[:, j, m * 128:(m + 1) * 128],
                                         fview[:, :, j],
                                         start=(j == 0), stop=False)
                for jt in range(4):
                    for m in range(4):
                        nc.tensor.matmul(psL[m][:],
                                         wtl[:, jt, m * 128:(m + 1) * 128],
                                         ftail[:, :, jt],
                                         start=False, stop=(jt == 3))
                for m in range(4):
                    nc.scalar.activation(a1[:, m, :], psL[m][:], AF.Relu,
                                         bias=b1s[:, m:m + 1],
                                         scale=sc1[:, m:m + 1])

            # ========== L23 / L4: 512x512 layers ===========================
            with tc.tile_pool(name="ps2m", bufs=4, space="PSUM") as ps2m:
                for m in range(4):
                    acc = ps2m.tile([128, BS], F32, tag="acc23")
                    for k in range(4):
                        nc.tensor.matmul(acc[:],
                                         w23s[:, k, m * 128:(m + 1) * 128],
                                         a1[:, k, :],
                                         start=(k == 0), stop=(k == 3))
                    nc.scalar.activation(h1[:, m, :], acc[:], AF.Relu,
                                         bias=b23s[:, m:m + 1], scale=GAM2)
                for m in range(4):
                    acc = ps2m.tile([128, BS], F32, tag="acc4")
                    for k in range(4):
                        nc.tensor.matmul(acc[:],
                                         w4s[:, k, m * 128:(m + 1) * 128],
                                         h1[:, k, :],
                                         start=(k == 0), stop=(k == 3))
                    nc.scalar.activation(h2[:, m, :], acc[:], AF.Relu,
                                         bias=b4s[:, m:m + 1],
                                         scale=sc4[:, m:m + 1])

            # ========== L5: out = h2 @ G + gb (ifft2 pre-folded) ===========
            with tc.tile_pool(name="ps5m", bufs=8, space="PSUM") as ps5m:
                for c in range(NCH):
                    acc = ps5m.tile([128, BS], F32, tag="acc5")
                    for k in range(4):
                        nc.tensor.matmul(acc[:], gs[:, k, c, :], h2[:, k, :],
                                         start=(k == 0), stop=False)
                    nc.tensor.matmul(acc[:],
                                     gbs[0:1, c * 128:(c + 1) * 128],
                                     ones1, start=False, stop=True)
                    if c % 2 == 0:
                        nc.scalar.copy(stage[:, c, :], acc[:])
                    else:
                        nc.vector.tensor_copy(stage[:, c, :], acc[:])
                    lo = {63: 48, 79: 64, 87: 80, 95: 88}
                    if c in (15, 31, 47):
                        c0 = c - 15
                        nc.sync.dma_start(d_out[:, c0:c + 1, :],
                                          stage[:, c0:c + 1, :])
                    elif c in lo:
                        c0 = lo[c]
                        nc.sync.dma_start(d_out[:, c0:c + 1, :],
                                          stage[:, c0:c + 1, :])

    nc.compile()
    return nc


def _get_nc():
    key = (W1_FP8, G_FP8)
    if key not in _NC_CACHE:
        _NC_CACHE[key] = _build_nc()
    return _NC_CACHE[key]


def _make_in_maps(x, Ws1, bs1, Ws2, bs2, Wn1, bn1, Wn2, bn2, Wn3, bn3):
    shared = dict(_dft_consts())
    shared.update(_prep_weights(Ws1, bs1, Ws2, bs2, Wn1, bn1, Wn2, bn2,
                                Wn3, bn3))
    in_maps = []
    for g in range(NCORE):
        if g == 0:
            halo = np.zeros((1, 64, 64), np.float32)
        else:
            halo = x[g * BS - 1, 2][None]
        ximgs = np.concatenate(
            [halo, x[g * BS:(g + 1) * BS].reshape(NIMG, 64, 64),
             np.zeros((1, 64, 64), np.float32)]).astype(np.float16)
        in_maps.append({"ximgs": np.ascontiguousarray(
            ximgs.transpose(1, 0, 2)), **shared})
    return in_maps


def kernel(**inputs):
    x = np.ascontiguousarray(inputs["x"], dtype=np.float32)
    nc = _get_nc()
    in_maps = _make_in_maps(
        x, inputs["Ws1"], inputs["bs1"], inputs["Ws2"], inputs["bs2"],
        inputs["Wn1"], inputs["bn1"], inputs["Wn2"], inputs["bn2"],
        inputs["Wn3"], inputs["bn3"])
    res = run_bass_kernel_spmd(nc, in_maps, list(range(NCORE)))
    out = np.empty((B, C, H, W), np.float32)
    for g in range(NCORE):
        st = np.asarray(res.results[g]["out"], dtype=np.float32) / GAM3
        out[g * BS:(g + 1) * BS] = st.transpose(2, 1, 0).reshape(
            BS, C, H, W)
    return out
